# revision 23
# baseline (speedup 1.0000x reference)
"""Trainium2 Bass kernel for nn_Conv2dLocal_47132971106931.

The reference module unfolds (1,128,256,256) -> (1, C*9, L), permutes and
*raw-reshapes* to (1, C, L, 9), multiplies by per-location weights (L, 9)
and sums the tap axis.  The raw reshape scrambles indices; the true math is

  out[0,c,y,x] = sum_k xpad[x%128, 2c+s+i_k, ((2y+t)%256)+j_k] * w[y*256+x, k]
      s = [y>=128], t = [x>=128], (i_k,j_k) = divmod(k,3)

In permuted coordinates  r' = 2c+s (0..255), q' = 2y'+t (0..255, y'=y%128),
a = x%128  this is a clean 3x3 local convolution over (r',q') with a
"batch" dim a:

  O[r',q',a] = sum_k xpad[a, r'+i, q'+j] * w[l(r'%2, q', a), k]
  l = (r'%2)*32768 + (q'//2)*256 + (q'%2)*128 + a

Sharding: q' is split across the 8 cores (32 columns each + 2 halo).
Device layout: partitions = a (128), free dims = (u = r'//2, q'-local).

Two engines split the u rows:

* DVE (rows u < DV): fp16 tensor_tensor passes (9 mul + 8 add) at the
  2x_1P perf mode.  The j=1 taps would be 2-byte-misaligned (mode falls
  back to 1x), so the host supplies a second input slab shifted by one
  column (xo) and j=1 taps read it at offset 0.
* PE (rows u >= DV): for each (s, q', k) a matmul with a *diagonal*
  stationary  D[a',m] = delta(a'==m) * w[l(s,q',a'),k]  computes the tap
  product (per-partition scale), and PSUM accumulation over the 9 taps
  does the adds for free (in fp32).  The 576 diagonal stationaries are
  built on the host and DMA'd once - the timed rep loop never reloads
  them.  PSUM banks hold 8 q'-columns x 64 u values; each finished bank
  is DMA'd straight to HBM (fp32).
"""

import sys
import numpy as np

TRN_REPO = "/opt/trn_rl_repo"

# ---- problem geometry (hardcoded) ----
C = 128
H = W = 256
NCORES = 8
QS = 32          # q' columns per core
QH = QS + 2      # with halo
NR = 258         # padded r' rows
NU = 128         # u = r'//2
DV = 128         # u rows computed on DVE; PE takes NU - DV
NPE = NU - DV
QG = 8           # q' columns per PSUM bank (QG * NPE = 512 fp32 = one bank)
# u rows (out of DV) handed to GPSIMD; 0 disables GPSIMD
GP_ROWS = 0
PS_BUFS = 2

# tap visit order for DVE: j=1 taps (k=1,4,7) last (their slab's DMA is last)
TAP_ORDER = (0, 2, 3, 5, 6, 8, 1, 4, 7)

_CACHE = {}


def _build_nc(dv=DV, gp_rows=GP_ROWS, reps=1):
    sys.path.insert(0, TRN_REPO)
    import concourse.bass as bass
    import concourse.mybir as mybir
    from concourse.bass import MemorySpace
    from concourse.tile import TileContext
    from concourse.tile_rust import add_dep_helper
    import bass_rust

    f16 = mybir.dt.float16
    f32 = mybir.dt.float32
    npe = NU - dv
    nbank = QS // QG          # PSUM banks per s half
    nc = bass.Bass("TRN2", target_bir_lowering=False, debug=False)

    xe_in = nc.dram_tensor("xe", (C, NR * QH), f16, kind="ExternalInput").ap()
    w_in = nc.dram_tensor("w", (C, 9 * 2 * QS), f16, kind="ExternalInput").ap()
    xo_in = nc.dram_tensor("xo", (C, NR * QH), f16, kind="ExternalInput").ap()
    dg_in = (nc.dram_tensor("dg", (C, 2 * QS * 9 * C), f16,
                            kind="ExternalInput").ap() if npe > 0 else None)
    yd_out = nc.dram_tensor("yd", (C, 2, dv, QS), f16, kind="ExternalOutput").ap()
    yp_out = (nc.dram_tensor(
        "yp", (C, 2, nbank, QG * npe), f16, kind="ExternalOutput"
    ).ap() if npe > 0 else None)

    def bcast_u(ap2, n):
        # [P, 32] -> [P, (u:0,n), 32]
        pstride, pcount = ap2.ap[0]
        qstride, qcount = ap2.ap[1]
        return bass_rust.AP(
            ap2.tensor, ap2.offset,
            bass_rust.VecI64Pair([[pstride, pcount], [0, n], [qstride, qcount]]),
        )

    with TileContext(nc) as tc:
        with (
            tc.tile_pool(name="xt", bufs=1) as xpool,
            tc.tile_pool(name="wt", bufs=1) as wpool,
            tc.tile_pool(name="dgp", bufs=1) as dgpool,
            tc.tile_pool(name="acc", bufs=2) as apool,
            tc.tile_pool(name="tmp", bufs=1) as tpool,
            tc.tile_pool(name="ps", bufs=PS_BUFS, space=MemorySpace.PSUM) as ppool,
            tc.tile_pool(name="psd", bufs=1, space=MemorySpace.PSUM) as pdpool,
            tc.tile_pool(name="stg", bufs=2) as spool,
        ):
            all_dmas = []
            xt = xpool.tile([C, NR * QH], f16, tag="xe")
            all_dmas.append(nc.sync.dma_start(out=xt[:, :], in_=xe_in[:, :]))

            wt = wpool.tile([C, 9 * 2 * QS], f16)
            all_dmas.append(nc.sync.dma_start(out=wt[:, :], in_=w_in[:, :]))
            w4 = wt.rearrange("p (k s c) -> p k s c", k=9, s=2, c=QS)

            xo = xpool.tile([C, NR * QH], f16, tag="xo")
            all_dmas.append(nc.sync.dma_start(out=xo[:, :], in_=xo_in[:, :]))

            if npe > 0:
                dgt = dgpool.tile([C, 2 * QS * 9 * C], f16)
                all_dmas.append(nc.sync.dma_start(out=dgt[:, :], in_=dg_in[:, :]))
                dg5 = dgt.rearrange("p (s q k m) -> p s q k m",
                                    s=2, q=QS, k=9, m=C)

            # u-pair view of the input rows: row = 2u + t
            xe4 = xt.rearrange("p (u t c) -> p u t c", u=129, t=2, c=QH)
            xo4 = xo.rearrange("p (u t c) -> p u t c", u=129, t=2, c=QH)

            pdummy = pdpool.tile([1, 2], f32)
            pdummy2 = spool.tile([1, 2], f16, tag="sd")

            _CHAIN = {"prev": None}
            prev_acc = {}
            did_pe_dummy = {}
            for rep in range(reps):
                rep_dmas = []
                # ---------------- PE half: rows [dv, 128) ----------------
                if npe > 0:
                    if not did_pe_dummy:
                        # 1x1 dummy matmuls absorb the dg- and xe-DMA sem
                        # ticks into the PE clock (walrus allows only one
                        # hardware sync wait per Matmult encoding).
                        did_pe_dummy["done"] = True
                        nc.tensor.matmul(
                            pdummy[0:1, 0:1], dgt[:, 0:1], dgt[:, 1:2],
                            start=True, stop=True,
                        )
                        nc.tensor.matmul(
                            pdummy[0:1, 1:2], xt[:, 0:1], xt[:, 1:2],
                            start=True, stop=True,
                        )
                    for s in range(2):
                        for qg in range(nbank):
                            gidx = _CHAIN.get("gidx", 0)
                            _CHAIN["gidx"] = gidx + 1
                            ptile = ppool.tile([C, QG * npe], f32, tag="pe")
                            last_mm = None
                            for qi in range(QG):
                                q = qg * QG + qi
                                for k in range(9):
                                    i, j = divmod(k, 3)
                                    dlt, rho = divmod(s + i, 2)
                                    rhs = xe4[:, dv + dlt:dv + dlt + npe,
                                              rho, q + j:q + j + 1]
                                    last_mm = nc.tensor.matmul(
                                        ptile[:, qi * npe:(qi + 1) * npe],
                                        dg5[:, s, q, k, :],
                                        rhs,
                                        start=(k == 0), stop=(k == 8),
                                        skip_group_check=True,
                                    )
                            # DMA cannot read PSUM: the (otherwise idle) ACT
                            # engine evacuates the bank to SBUF as fp16, then
                            # the DMA stores it. A 1-element ACT dummy write
                            # into the stage buf absorbs the WAR vs the
                            # previous store-DMA (Activation encodings carry
                            # two sync waits; Matmult/TensorTensor only one).
                            stage = spool.tile([C, QG * npe], f16, tag="st")
                            nc.scalar.copy(stage[0:1, 0:1], wt[0:1, 0:1])
                            cp = nc.scalar.copy(stage[:, :], ptile[:, :])
                            # ACT takes ownership of the spent bank: rewrite
                            # it wholesale (garbage; start=True resets it) so
                            # the NEXT group's first matmul sees a single ACT
                            # last-writer and needs only one Act-sem wait
                            # instead of Act-WAR + PE-self-sem (two waits,
                            # which the Matmult encoding cannot carry).
                            zp = nc.scalar.copy(ptile[:, :], stage[:, :])
                            # Pre-credit the PE clock with this evacuation's
                            # Act tick: a value-preserving 1-element self-copy
                            # into the dg slice that a MID-group Ldweights of
                            # the next group reads. That Ldweights (auto RAW)
                            # carries the Act wait well before the conflicted
                            # group-first matmul, and mid-group placement
                            # avoids stalling the next group's start.
                            ng = gidx + 1
                            if ng < reps * 2 * nbank:
                                g_in_rep = ng % (2 * nbank)
                                s_n, qg_n = divmod(g_in_rep, nbank)
                                q_mid = qg_n * QG + QG // 2
                                el = dg5[0:1, s_n, q_mid, 0, 0:1]
                                nc.scalar.copy(pdummy2[0:1, 0:1], el)
                                nc.scalar.copy(el, pdummy2[0:1, 0:1])
                            drn = nc.sync.drain()
                            add_dep_helper(drn.ins, cp.ins, sync=True,
                                           reason="absorb ACT tick")
                            dma = nc.sync.dma_start(
                                out=yp_out[:, s, qg, :], in_=stage[:, :]
                            )
                            add_dep_helper(dma.ins, drn.ins, sync=False,
                                           reason="keep drain before out-dma")
                            rep_dmas.append(dma)
                # ---------------- DVE/GPSIMD half: rows [0, dv) ----------
                for s in range(2):
                    nd = dv - gp_rows
                    splits = []
                    if nd > 0:
                        splits.append(("v", 0, nd))
                    if gp_rows > 0:
                        splits.append(("g", nd, dv))
                    for eng_name, a0, a1 in splits:
                        n = a1 - a0
                        eng = nc.vector if eng_name == "v" else nc.gpsimd
                        acc = apool.tile([C, n * QS], f16, tag=f"acc{eng_name}")
                        acc3 = acc.rearrange("p (u c) -> p u c", u=n, c=QS)
                        tmp = tpool.tile([C, n * QS], f16, tag=f"tmp{eng_name}")
                        tmp3 = tmp.rearrange("p (u c) -> p u c", u=n, c=QS)
                        key = (s, eng_name)
                        if key in prev_acc:
                            # rep>0: read the previous rep's acc so every
                            # rep's compute is live up to the final store.
                            eng.tensor_copy(tmp[:, 0:1], prev_acc[key][:, 0:1])
                        else:
                            # dummy 1-elem reads absorb the xo- and w-DMA sem
                            # ticks into this engine's observed clock. They
                            # write into tmp (NOT acc): a write to acc would
                            # WAW-feed the k=0 mul, and that dep lowers to a
                            # self-sem wait which together with the mul's
                            # xe-DMA wait exceeds the TensorTensor encoding's
                            # single sync-wait slot. The WAW lands on the
                            # idx=1 mul instead, which carries no DMA wait.
                            eng.tensor_copy(tmp[:, 0:1], xo[:, 0:1])
                            eng.tensor_copy(tmp[:, 1:2], wt[:, 0:1])
                        prev_acc[key] = acc
                        last = None
                        for idx, k in enumerate(TAP_ORDER):
                            i, j = divmod(k, 3)
                            dlt, rho = divmod(s + i, 2)
                            src = xe4 if j != 1 else xo4
                            jj = j if j != 1 else 0
                            in0 = src[:, a0 + dlt:a0 + dlt + n, rho, jj:jj + QS]
                            in1 = bcast_u(w4[:, k, s, :], n)
                            if idx == 0:
                                last = eng.tensor_mul(acc3[:, :, :], in0, in1)
                            else:
                                eng.tensor_mul(tmp3[:, :, :], in0, in1)
                                last = eng.tensor_add(
                                    acc3[:, :, :], acc3[:, :, :], tmp3[:, :, :]
                                )
                        if rep != reps - 1:
                            continue
                        # Sequencer drain absorbs the producer-engine tick
                        # into the sync engine, so the out-DMA needs at most
                        # its own-queue wait.
                        drn = nc.sync.drain()
                        add_dep_helper(drn.ins, last.ins, sync=True,
                                       reason="absorb producer tick")
                        dma = nc.sync.dma_start(
                            out=yd_out[:, s, a0:a1, :], in_=acc3[:, :, :]
                        )
                        add_dep_helper(dma.ins, drn.ins, sync=False,
                                       reason="keep drain before out-dma")
                        rep_dmas.append(dma)
                # Absorb: one 1-wait drain per DMA so the sync engine
                # observes every DMA-queue sem (the auto-emitted kernel
                # tail drain would otherwise need one wait per queue and
                # exceed its sync-wait slots).
                for d in (all_dmas if rep == 0 else []) + rep_dmas:
                    ad = nc.sync.drain()
                    add_dep_helper(ad.ins, d.ins, sync=True,
                                   reason="dma absorb")
                    if _CHAIN.get("prev") is not None:
                        add_dep_helper(ad.ins, _CHAIN["prev"].ins, sync=False,
                                       reason="order absorb drains")
                    _CHAIN["prev"] = ad
    return nc


def _get_nc(reps=1):
    key = ("nc", DV, GP_ROWS, reps)
    if key not in _CACHE:
        _CACHE[key] = _build_nc(reps=reps)
    return _CACHE[key]


def _prep_inputs(input_tensor, weights):
    x = np.ascontiguousarray(np.asarray(input_tensor, dtype=np.float32))
    w = np.ascontiguousarray(np.asarray(weights, dtype=np.float32))
    # pad one extra column on the right so the odd-shifted slab exists for
    # the last core (cols q0+1 .. q0+34 with q0 = 224)
    xp = np.pad(x[0], ((0, 0), (1, 1), (1, 2))).astype(np.float16)  # (128,258,259)

    a = np.arange(C)
    in_maps = []
    for m in range(NCORES):
        q0 = QS * m
        xe = np.ascontiguousarray(xp[:, :, q0:q0 + QH]).reshape(C, NR * QH)
        xo = np.ascontiguousarray(xp[:, :, q0 + 1:q0 + 1 + QH]).reshape(C, NR * QH)
        # W[a, k, s, ql] = w[l, k],  l = s*32768 + ((q0+ql)//2)*256 + ((q0+ql)%2)*128 + a
        ql = np.arange(QS)
        yq = (q0 + ql) // 2
        tq = (q0 + ql) % 2
        s_ = np.arange(2)
        l_idx = (s_[:, None, None] * 32768
                 + (yq * 256 + tq * 128)[None, :, None]
                 + a[None, None, :])              # (2, QS, 128)
        wm = w[l_idx]                             # (2, QS, 128, 9)
        wmk = np.ascontiguousarray(
            np.transpose(wm, (2, 3, 0, 1)).reshape(C, 9 * 2 * QS)
        ).astype(np.float16)
        # diagonal stationaries dg[a', s, q, k, m] = delta(a'==m) * wm[s,q,a',k]
        dg = np.zeros((C, 2, QS, 9, C) if NPE > 0 else (1,), np.float16)
        if NPE > 0:
            wa = np.transpose(wm, (2, 0, 1, 3)).astype(np.float16)  # (a,s,q,k)
            ar = np.arange(C)
            dg[ar[:, None, None, None], s_[None, :, None, None],
               ql[None, None, :, None], np.arange(9)[None, None, None, :],
               ar[:, None, None, None]] = wa
        im = {"xe": xe, "w": wmk, "xo": xo}
        if NPE > 0:
            im["dg"] = np.ascontiguousarray(dg.reshape(C, 2 * QS * 9 * C))
        in_maps.append(im)
    return in_maps


def _gather_output(results):
    nbank = QS // QG
    out = np.empty((C, H, W), np.float32)
    for m in range(NCORES):
        q0 = QS * m
        yd = results[m]["yd"].astype(np.float32).reshape(C, 2, DV, QS)
        dev = np.empty((C, 2, NU, QS), np.float32)      # [a, s, u, ql]
        dev[:, :, :DV, :] = yd
        if NPE > 0:
            yp = results[m]["yp"].reshape(C, 2, nbank, QG, NPE)
            # yp[a, s, qg, qi, n] = O at (u = DV+n, ql = qg*QG+qi)
            dev[:, :, DV:, :] = np.transpose(yp, (0, 1, 4, 2, 3)).reshape(
                C, 2, NPE, QS)
        # out[u, s*128 + (q0+ql)//2, ((q0+ql)%2)*128 + a] = dev[a, s, u, ql]
        d = dev.reshape(C, 2, NU, QS // 2, 2)        # [a, s, u, v, t]
        d = np.transpose(d, (2, 1, 3, 4, 0))         # [u, s, v, t, a]
        y0 = q0 // 2
        for s in range(2):
            for t in range(2):
                out[:, s * 128 + y0: s * 128 + y0 + QS // 2,
                    t * 128: t * 128 + C] = d[:, s, :, t, :]
    return out.reshape(1, C, H, W)


def _run(in_maps, trace=False):
    sys.path.insert(0, TRN_REPO)
    from concourse.bass_utils import run_bass_kernel_spmd

    nc = _get_nc()
    res = run_bass_kernel_spmd(
        nc, in_maps, core_ids=list(range(NCORES)), trace=trace
    )
    return res


def kernel(input_tensor, weights):
    in_maps = _prep_inputs(input_tensor, weights)
    res = _run(in_maps, trace=False)
    return _gather_output(res.results)


def bench(input_tensor, weights, trace=True):
    in_maps = _prep_inputs(input_tensor, weights)
    res = _run(in_maps, trace=trace)
    return _gather_output(res.results), res


def _make_runner(nc, in_maps):
    """Build a reusable jitted 8-core runner for a prebuilt nc.
    Returns (call, gather) where call() executes once and returns device
    outputs, gather(outs) -> per-core result dicts."""
    sys.path.insert(0, TRN_REPO)
    import jax
    import numpy as np_
    from jax.sharding import Mesh, PartitionSpec
    from jax.experimental.shard_map import shard_map
    from concourse import bass2jax
    import concourse.mybir as mybir

    bass2jax.install_neuronx_cc_hook()

    partition_name = (
        nc.partition_id_tensor.name if nc.partition_id_tensor else None
    )
    in_names, out_names, out_avals, zero_outs = [], [], [], []
    for alloc in nc.m.functions[0].allocations:
        if not isinstance(alloc, mybir.MemoryLocationSet):
            continue
        name = alloc.memorylocations[0].name
        if alloc.kind == "ExternalInput":
            if name != partition_name:
                in_names.append(name)
        elif alloc.kind == "ExternalOutput":
            shape = tuple(alloc.tensor_shape)
            dtype = mybir.dt.np(alloc.dtype)
            out_avals.append(jax.core.ShapedArray(shape, dtype))
            out_names.append(name)
            zero_outs.append(np_.zeros(shape, dtype))
    n_params = len(in_names)
    n_outs = len(out_names)
    all_in_names = list(in_names) + list(out_names)
    if partition_name is not None:
        all_in_names.append(partition_name)

    def _body(*args):
        ins = list(args[:n_params])
        outs = list(args[n_params:])
        pid = [bass2jax.partition_id_tensor()] if partition_name else []
        outs = list(bass2jax._bass_exec_p.bind(
            *ins, *outs, *pid,
            out_avals=tuple(out_avals),
            in_names=tuple(all_in_names),
            out_names=tuple(out_names),
            lowering_input_output_aliases=(),
            sim_require_finite=True,
            sim_require_nnan=True,
            nc=nc,
        ))
        return tuple(outs)

    devices = jax.devices()[:NCORES]
    mesh = Mesh(np_.asarray(devices), ("core",))
    in_specs = (PartitionSpec("core"),) * (n_params + n_outs)
    out_specs = (PartitionSpec("core"),) * n_outs
    donate = tuple(range(n_params, n_params + n_outs))

    per_core = [[np_.asarray(m[nm]) for nm in in_names] for m in in_maps]
    concat_in = [
        np_.concatenate([per_core[c][i] for c in range(NCORES)], axis=0)
        for i in range(n_params)
    ]
    concat_zeros = [
        np_.zeros((NCORES * z.shape[0], *z.shape[1:]), z.dtype)
        for z in zero_outs
    ]

    f = jax.jit(
        shard_map(_body, mesh=mesh, in_specs=in_specs,
                  out_specs=out_specs, check_rep=False),
        donate_argnums=donate, keep_unused=True,
    )
    cin = [jax.device_put(a) for a in concat_in]
    state = {"outs": None}

    def call():
        prev = state["outs"]
        if prev is None:
            prev = [jax.device_put(z) for z in concat_zeros]
        outs = f(*cin, *prev)
        for o in outs:
            o.block_until_ready()
        state["outs"] = list(outs)
        return outs

    def gather(outs):
        return [
            {nm: np_.asarray(outs[i]).reshape(NCORES, *out_avals[i].shape)[c]
             for i, nm in enumerate(out_names)}
            for c in range(NCORES)
        ]

    return call, gather


def time_kernel(input_tensor, weights, k_long=11, reps=12):
    """Per-iteration device time via in-NEFF repetition: build the same
    program with the compute+store body repeated K times (inputs loaded
    once), then dt = (t_K - t_1) / (K - 1) cancels the proxy round-trip
    and NEFF launch overhead.

    Returns (dt_seconds, t1_seconds, out_full_from_k_run)."""
    import time as _time
    in_maps = _prep_inputs(input_tensor, weights)
    call1, gather1 = _make_runner(_get_nc(reps=1), in_maps)
    callk, gatherk = _make_runner(_get_nc(reps=k_long), in_maps)

    call1(); callk()  # compile + warm
    t1s, tks = [], []
    outs_k = None
    for _ in range(reps):
        t0 = _time.perf_counter()
        call1()
        t1s.append(_time.perf_counter() - t0)
        t0 = _time.perf_counter()
        outs_k = callk()
        tks.append(_time.perf_counter() - t0)
    dt = (min(tks) - min(t1s)) / (k_long - 1)
    print(f"[time_kernel] t1 samples (ms): {[round(t*1e3,2) for t in t1s]}")
    print(f"[time_kernel] t{k_long} samples (ms): {[round(t*1e3,2) for t in tks]}")
    return dt, min(t1s), _gather_output(gatherk(outs_k))
